# revision 4
# baseline (speedup 1.0000x reference)
"""HashEmbedding (hash -> gather -> sum-pool) on 8 TRN2 NeuronCores.

Strategy: batch-data-parallel (each core owns 512 of the 4096 batch rows
and a full fp16 copy of the [1M, 128] table in its local HBM). Gather
runs on the ANT `dma_gather` SWDGE primitive; pooling is TensorE matmul
with 0/1 assignment matrices built on the DVE.

v2 changes vs the 322us baseline (which was Pool-engine-bound on SWDGE
descriptor emission: 124 calls x ~2.4us):
  - ONE gather call per 32768-row window (31 calls instead of 124):
    the four 128-row batch groups' slots are concatenated group-major
    inside each window call, cutting per-call Pool overhead
    (dispatch + ring setup + reg loads + sem waits) ~4x.
  - fp16 table (halves gather HBM traffic to 26 MB/core; rel err from
    fp16 quantization of the summed N(0,1) values is ~1e-4, far inside
    the 2e-2 gate), fp16 assignment matrices (2x DVE is_equal rate),
    fp16 matmuls (PE fp16 rate >> its fp32 rate).
  - slot values are 128*group + row (0..511, exact in fp16); chunks that
    straddle a group boundary get one matmul per group they touch, with
    the per-group iota tile (iota + 128g) selecting only that group's
    slots. This keeps segment padding at roundup16 (not roundup128).
  - num_idxs passed as an immediate (= the build-time per-call cap, the
    max over cores): no per-call count reg_load on Pool (was 37.6us).
    Padded slots point at window row 0 (safe in-bounds read) and carry
    slot -1 so they match no assignment column.
  - windows issued largest-first so the final call (whose A-build +
    matmuls trail the last emission) is the smallest.
"""

import sys

if "/opt/trn_rl_repo" not in sys.path:
    sys.path.insert(0, "/opt/trn_rl_repo")

import numpy as np

B, H, D, V = 4096, 200, 128, 1_000_000
NCORES = 8
BPC = B // NCORES              # 512 batch rows per core
NPASS = 4                      # batch groups of 128 rows (PSUM M limit)
WBITS = 15
W = 1 << WBITS                 # 32768-row window (int16 index limit)
NW = (V + W - 1) // W          # 31 windows
NSEG = NW * NPASS              # 124 (window, group) segments
NQ = 4                         # SWDGE queues (ucode max 4)

_cache: dict = {}


def _hash_buckets(x_core):
    """Per-core (seg, loc, slot512): seg = window*NPASS + group,
    loc = row within window, slot512 = group*128 + batch-row-in-group."""
    idx = (
        (x_core.astype(np.uint32).ravel() * np.uint32(2654435761))
        % np.uint32(V)
    ).astype(np.int32)                       # [BPC*H]
    b = np.repeat(np.arange(BPC, dtype=np.int32), H)
    g = b >> 7
    seg = (idx >> WBITS) * NPASS + g
    return seg, (idx & (W - 1)), g * 128 + (b & 127)


def _plan(caps):
    """Build-time geometry shared by all cores (SPMD-uniform).

    caps: [NSEG] per-(window,group) slot capacity, multiple of 16.
    Returns dict with per-window slot offsets, chunk offsets, chunk
    counts, per-(w,g) chunk ranges, and the window issue order.
    """
    capw = caps.reshape(NW, NPASS)
    wsum = capw.sum(axis=1)                        # slots per window call
    segoff = np.zeros((NW, NPASS), dtype=np.int64)
    segoff[:, 1:] = np.cumsum(capw, axis=1)[:, :-1]
    chk = (wsum + 127) // 128                      # chunks per window
    order = np.argsort(-wsum, kind="stable")       # issue largest first
    woff = np.zeros(NW, dtype=np.int64)            # slot offset, issue order
    coff = np.zeros(NW, dtype=np.int64)            # chunk offset, issue order
    so = co = 0
    for i, w in enumerate(order):
        woff[w] = so
        coff[w] = co
        so += int(wsum[w])
        co += int(chk[w])
    cg0 = segoff // 128                            # first chunk of (w,g)
    cg1 = (segoff + capw + 127) // 128             # one past last chunk
    return dict(
        capw=capw, wsum=wsum, segoff=segoff, chk=chk, order=order,
        woff=woff, coff=coff, cg0=cg0, cg1=cg1,
        total=int(wsum.sum()), tchunks=int(chk.sum()),
    )


def _layout(seg, loc, slot, plan):
    """Per-core device tensors: wrapped loc16 [128, total//16] and
    slotf [128, tchunks] fp16."""
    order = np.argsort(seg, kind="stable")
    ss, ls, vs = seg[order], loc[order], slot[order]
    counts = np.bincount(seg, minlength=NSEG)
    starts = np.zeros(NSEG, dtype=np.int64)
    starts[1:] = np.cumsum(counts)[:-1]
    rank = np.arange(ss.size) - starts[ss]

    w_of = ss // NPASS
    g_of = ss % NPASS
    pos = plan["woff"][w_of] + plan["segoff"][w_of, g_of] + rank

    # All pads are index 0 (safe in-bounds read of window row 0; their
    # slotf is -1 so they match no assignment column). Never use -1 pads:
    # the ucode's trailing-(-1) trim would emit fewer ring descriptors
    # than the NX decode reserved, desyncing the SDMA ring pointer.
    total = plan["total"]
    flat_loc = np.zeros(total, dtype=np.int16)
    flat_loc[pos] = ls.astype(np.int16)

    slot_pad = np.full((128, plan["tchunks"]), -1.0, dtype=np.float16)
    within = pos - plan["woff"][w_of]              # position within call
    col = plan["coff"][w_of] + within // 128
    slot_pad[within % 128, col] = vs.astype(np.float16)

    wrapped = flat_loc.reshape(total // 16, 16).T          # [16, total//16]
    loc16 = np.tile(wrapped, (8, 1)).copy()                # [128, total//16]
    return loc16, slot_pad


def _build(caps, plan):
    import concourse.tile as tile
    from concourse import bacc, mybir

    i16, i32, f16, f32 = (
        mybir.dt.int16, mybir.dt.int32, mybir.dt.float16, mybir.dt.float32
    )
    Alu = mybir.AluOpType
    total_cols = plan["total"] // 16
    tchunks = plan["tchunks"]
    chkmax = int(plan["chk"].max())
    ncgmax = int((plan["cg1"] - plan["cg0"]).max())

    nc = bacc.Bacc(
        "TRN2",
        target_bir_lowering=False,
        debug=False,
        enable_asserts=False,
        # SWDGE ring: carveout_ndesc = scratch//16 per queue; a window call
        # needs ~wsum/16+1 descs per lane per side (~225), so 4096 gives
        # each queue ~18 calls of headroom.
        dynamic_dma_scratch_size=65536,
        num_swdge_queues=NQ,
    )
    tb_ap = nc.dram_tensor("table", [NW * W, D], f16, kind="ExternalInput").ap()
    loc_ap = nc.dram_tensor(
        "loc16", [128, total_cols], i16, kind="ExternalInput"
    ).ap()
    slot_ap = nc.dram_tensor(
        "slotf", [128, tchunks], f16, kind="ExternalInput"
    ).ap()
    out_ap = nc.dram_tensor("out", [BPC, D], f32, kind="ExternalOutput").ap()

    worder = [int(w) for w in plan["order"]]

    with tile.TileContext(nc) as tc:
        with (
            tc.tile_pool(name="iop", bufs=1) as iop,
            tc.tile_pool(name="gp", bufs=6) as gp,
            tc.tile_pool(name="ap_", bufs=12) as ap_,
            tc.tile_pool(name="op", bufs=2) as op,
            tc.tile_pool(name="pp", bufs=1, space="PSUM") as pp,
        ):
            # warmup gather first: warms the Q7 gather ucode (cold first
            # call otherwise costs ~10 us) while uploads run.
            iota_i = iop.tile([128, 128], i32, name="iota_i")
            nc.gpsimd.iota(iota_i[:], [[1, 128]], base=0, channel_multiplier=0)
            junk = iop.tile([128, 1, D], f16, name="junk")
            nc.gpsimd.dma_gather(
                junk[:],
                tb_ap[0:W, :],
                iota_i[:].bitcast(i16)[:, 0:1],
                16,
                16,
                D,
                single_packet=False,
                queue_num=3,
            )

            # per-group iota tiles: iota_g[p, m] = 128*g + m, fp16
            iotas = []
            for g in range(NPASS):
                t = iop.tile([128, 128], f16, name=f"iota{g}")
                if g == 0:
                    nc.vector.tensor_copy(t[:], iota_i[:])
                else:
                    nc.vector.tensor_scalar(
                        t[:], iotas[0][:], float(128 * g), None, Alu.add
                    )
                iotas.append(t)

            stall = iop.tile([128, tchunks], f16, name="stall")
            nc.sync.dma_start(out=stall[:], in_=slot_ap[:])

            # index upload split so the first call's columns land first
            ltall = iop.tile([128, total_cols], i16, name="ltall")
            head_cols = int(plan["wsum"][worder[0]]) // 16
            if 0 < head_cols < total_cols:
                nc.sync.dma_start(
                    out=ltall[:, :head_cols], in_=loc_ap[:, :head_cols]
                )
                nc.sync.dma_start(
                    out=ltall[:, head_cols:], in_=loc_ap[:, head_cols:]
                )
            else:
                nc.sync.dma_start(out=ltall[:], in_=loc_ap[:])

            psums = [
                pp.tile([128, D], f32, name=f"ps{g}", tag=f"ps{g}")
                for g in range(NPASS)
            ]

            for i, w in enumerate(worder):
                wsum_w = int(plan["wsum"][w])
                chk_w = int(plan["chk"][w])
                col0 = int(plan["woff"][w]) // 16

                g_t = gp.tile([128, chkmax, D], f16, name="g", tag="g")
                nc.gpsimd.dma_gather(
                    g_t[:, :chk_w, :],
                    tb_ap[w * W : (w + 1) * W, :],
                    ltall[:, col0 : col0 + wsum_w // 16],
                    wsum_w,
                    wsum_w,
                    D,
                    single_packet=False,
                    queue_num=i % NQ,
                )

                ccol0 = int(plan["coff"][w])
                for g in range(NPASS):
                    c0 = int(plan["cg0"][w, g])
                    c1 = int(plan["cg1"][w, g])
                    ncg = c1 - c0
                    A = ap_.tile([128, ncgmax, 128], f16, name="A", tag="A")
                    iota_bc = iotas[g][:].unsqueeze(1).broadcast_to(
                        [128, ncg, 128]
                    )
                    st_bc = stall[:, ccol0 + c0 : ccol0 + c1].unsqueeze(
                        2
                    ).broadcast_to([128, ncg, 128])
                    nc.vector.tensor_tensor(
                        A[:, :ncg, :], iota_bc, st_bc, Alu.is_equal
                    )
                    for c in range(ncg):
                        nc.tensor.matmul(
                            psums[g][:],
                            A[:, c, :],
                            g_t[:, c0 + c, :],
                            start=(i == 0 and c == 0),
                            stop=(i == NW - 1 and c == ncg - 1),
                        )

            for g in range(NPASS):
                outs = op.tile([128, D], f32, name="outs", tag="outs")
                nc.vector.tensor_copy(outs[:], psums[g][:])
                nc.sync.dma_start(
                    out=out_ap[g * 128 : (g + 1) * 128, :], in_=outs[:]
                )

    nc.compile()
    return nc


def _run(x, table, trace=False):
    from concourse.bass_utils import run_bass_kernel_spmd

    x_np = np.asarray(x)
    per_core = [
        _hash_buckets(x_np[c * BPC : (c + 1) * BPC]) for c in range(NCORES)
    ]
    cmax = np.max(
        [np.bincount(s, minlength=NSEG) for s, _, _ in per_core], axis=0
    )
    caps = (((np.maximum(cmax, 1) + 15) // 16) * 16).astype(np.int64)
    plan = _plan(caps)

    if "nc" not in _cache:
        _cache["nc"] = _build(caps, plan)
    nc = _cache["nc"]

    # fp16 table padded to NW*W rows so every gather window is full
    tb = np.zeros((NW * W, D), dtype=np.float16)
    tb[:V] = np.asarray(table).astype(np.float16)
    in_maps = []
    for c in range(NCORES):
        loc16, slotf = _layout(*per_core[c], plan)
        in_maps.append({"table": tb, "loc16": loc16, "slotf": slotf})
    res = run_bass_kernel_spmd(nc, in_maps, list(range(NCORES)), trace=trace)
    out = np.concatenate(
        [res.results[c]["out"] for c in range(NCORES)], axis=0
    ).astype(np.float32)
    return out, res


def kernel(x, table):
    out, _ = _run(x, table, trace=False)
    return out


# revision 5
# speedup vs baseline: 1.0388x; 1.0388x over previous
"""HashEmbedding (hash -> gather -> sum-pool) on 8 TRN2 NeuronCores.

Strategy: batch-data-parallel (each core owns 512 of the 4096 batch rows
and a full fp16 copy of the [1M, 128] table in its local HBM). Gather
runs on the ANT `dma_gather` SWDGE primitive; pooling is TensorE matmul
with 0/1 assignment matrices built on the DVE.

v2 changes vs the 322us baseline (which was Pool-engine-bound on SWDGE
descriptor emission: 124 calls x ~2.4us):
  - ONE gather call per 32768-row window (31 calls instead of 124):
    the four 128-row batch groups' slots are concatenated group-major
    inside each window call, cutting per-call Pool overhead
    (dispatch + ring setup + reg loads + sem waits) ~4x.
  - fp16 table (halves gather HBM traffic to 26 MB/core; rel err from
    fp16 quantization of the summed N(0,1) values is ~1e-4, far inside
    the 2e-2 gate), fp16 assignment matrices (2x DVE is_equal rate),
    fp16 matmuls (PE fp16 rate >> its fp32 rate).
  - slot values are 128*group + row (0..511, exact in fp16); chunks that
    straddle a group boundary get one matmul per group they touch, with
    the per-group iota tile (iota + 128g) selecting only that group's
    slots. This keeps segment padding at roundup16 (not roundup128).
  - num_idxs passed as an immediate (= the build-time per-call cap, the
    max over cores): no per-call count reg_load on Pool (was 37.6us).
    Padded slots point at window row 0 (safe in-bounds read) and carry
    slot -1 so they match no assignment column.
  - windows issued largest-first so the final call (whose A-build +
    matmuls trail the last emission) is the smallest.
"""

import sys

if "/opt/trn_rl_repo" not in sys.path:
    sys.path.insert(0, "/opt/trn_rl_repo")

import numpy as np

B, H, D, V = 4096, 200, 128, 1_000_000
NCORES = 8
BPC = B // NCORES              # 512 batch rows per core
NPASS = 4                      # batch groups of 128 rows (PSUM M limit)
WBITS = 15
W = 1 << WBITS                 # 32768-row window (int16 index limit)
NW = (V + W - 1) // W          # 31 windows
NSEG = NW * NPASS              # 124 (window, group) segments
NQ = 4                         # SWDGE queues (ucode max 4)

_cache: dict = {}


def _hash_buckets(x_core):
    """Per-core (seg, loc, slot512): seg = window*NPASS + group,
    loc = row within window, slot512 = group*128 + batch-row-in-group."""
    idx = (
        (x_core.astype(np.uint32).ravel() * np.uint32(2654435761))
        % np.uint32(V)
    ).astype(np.int32)                       # [BPC*H]
    b = np.repeat(np.arange(BPC, dtype=np.int32), H)
    g = b >> 7
    seg = (idx >> WBITS) * NPASS + g
    return seg, (idx & (W - 1)), g * 128 + (b & 127)


def _plan(caps):
    """Build-time geometry shared by all cores (SPMD-uniform).

    caps: [NSEG] per-(window,group) slot capacity, multiple of 16.
    Returns dict with per-window slot offsets, chunk offsets, chunk
    counts, per-(w,g) chunk ranges, and the window issue order.
    """
    capw = caps.reshape(NW, NPASS)
    wsum = capw.sum(axis=1)                        # slots per window call
    segoff = np.zeros((NW, NPASS), dtype=np.int64)
    segoff[:, 1:] = np.cumsum(capw, axis=1)[:, :-1]
    chk = (wsum + 127) // 128                      # chunks per window
    order = np.argsort(-wsum, kind="stable")       # issue largest first
    woff = np.zeros(NW, dtype=np.int64)            # slot offset, issue order
    coff = np.zeros(NW, dtype=np.int64)            # chunk offset, issue order
    so = co = 0
    for i, w in enumerate(order):
        woff[w] = so
        coff[w] = co
        so += int(wsum[w])
        co += int(chk[w])
    cg0 = segoff // 128                            # first chunk of (w,g)
    cg1 = (segoff + capw + 127) // 128             # one past last chunk
    return dict(
        capw=capw, wsum=wsum, segoff=segoff, chk=chk, order=order,
        woff=woff, coff=coff, cg0=cg0, cg1=cg1,
        total=int(wsum.sum()), tchunks=int(chk.sum()),
    )


def _layout(seg, loc, slot, plan):
    """Per-core device tensors: wrapped loc16 [128, total//16] and
    slotf [128, tchunks] fp16."""
    order = np.argsort(seg, kind="stable")
    ss, ls, vs = seg[order], loc[order], slot[order]
    counts = np.bincount(seg, minlength=NSEG)
    starts = np.zeros(NSEG, dtype=np.int64)
    starts[1:] = np.cumsum(counts)[:-1]
    rank = np.arange(ss.size) - starts[ss]

    w_of = ss // NPASS
    g_of = ss % NPASS
    pos = plan["woff"][w_of] + plan["segoff"][w_of, g_of] + rank

    # All pads are index 0 (safe in-bounds read of window row 0; their
    # slotf is -1 so they match no assignment column). Never use -1 pads:
    # the ucode's trailing-(-1) trim would emit fewer ring descriptors
    # than the NX decode reserved, desyncing the SDMA ring pointer.
    total = plan["total"]
    flat_loc = np.zeros(total, dtype=np.int16)
    flat_loc[pos] = ls.astype(np.int16)

    slot_pad = np.full((128, plan["tchunks"]), -1.0, dtype=np.float16)
    within = pos - plan["woff"][w_of]              # position within call
    col = plan["coff"][w_of] + within // 128
    slot_pad[within % 128, col] = vs.astype(np.float16)

    wrapped = flat_loc.reshape(total // 16, 16).T          # [16, total//16]
    loc16 = np.tile(wrapped, (8, 1)).copy()                # [128, total//16]
    return loc16, slot_pad


def _build(caps, plan):
    import concourse.tile as tile
    from concourse import bacc, mybir

    i16, i32, f16, f32 = (
        mybir.dt.int16, mybir.dt.int32, mybir.dt.float16, mybir.dt.float32
    )
    Alu = mybir.AluOpType
    total_cols = plan["total"] // 16
    tchunks = plan["tchunks"]
    chkmax = int(plan["chk"].max())
    ncgmax = int((plan["cg1"] - plan["cg0"]).max())

    nc = bacc.Bacc(
        "TRN2",
        target_bir_lowering=False,
        debug=False,
        enable_asserts=False,
        # SWDGE ring: carveout_ndesc = scratch//16 per queue; a window call
        # needs ~wsum/16+1 descs per lane per side (~225), so 2048 gives
        # each queue ~9 calls of headroom. Deep A/G pools let the DVE
        # pre-build assignment matrices so the final windows' matmuls
        # don't serialize behind their gather drains (tail shrink).
        dynamic_dma_scratch_size=32768,
        num_swdge_queues=NQ,
    )
    tb_ap = nc.dram_tensor("table", [NW * W, D], f16, kind="ExternalInput").ap()
    loc_ap = nc.dram_tensor(
        "loc16", [128, total_cols], i16, kind="ExternalInput"
    ).ap()
    slot_ap = nc.dram_tensor(
        "slotf", [128, tchunks], f16, kind="ExternalInput"
    ).ap()
    out_ap = nc.dram_tensor("out", [BPC, D], f32, kind="ExternalOutput").ap()

    worder = [int(w) for w in plan["order"]]

    with tile.TileContext(nc) as tc:
        with (
            tc.tile_pool(name="iop", bufs=1) as iop,
            tc.tile_pool(name="gp", bufs=8) as gp,
            tc.tile_pool(name="ap_", bufs=24) as ap_,
            tc.tile_pool(name="op", bufs=2) as op,
            tc.tile_pool(name="pp", bufs=1, space="PSUM") as pp,
        ):
            # warmup gather first: warms the Q7 gather ucode (cold first
            # call otherwise costs ~10 us) while uploads run.
            iota_i = iop.tile([128, 128], i32, name="iota_i")
            nc.gpsimd.iota(iota_i[:], [[1, 128]], base=0, channel_multiplier=0)
            junk = iop.tile([128, 1, D], f16, name="junk")
            nc.gpsimd.dma_gather(
                junk[:],
                tb_ap[0:W, :],
                iota_i[:].bitcast(i16)[:, 0:1],
                16,
                16,
                D,
                single_packet=False,
                queue_num=3,
            )

            # per-group iota tiles: iota_g[p, m] = 128*g + m, fp16
            iotas = []
            for g in range(NPASS):
                t = iop.tile([128, 128], f16, name=f"iota{g}")
                if g == 0:
                    nc.vector.tensor_copy(t[:], iota_i[:])
                else:
                    nc.vector.tensor_scalar(
                        t[:], iotas[0][:], float(128 * g), None, Alu.add
                    )
                iotas.append(t)

            stall = iop.tile([128, tchunks], f16, name="stall")
            nc.sync.dma_start(out=stall[:], in_=slot_ap[:])

            # index upload split so the first call's columns land first
            ltall = iop.tile([128, total_cols], i16, name="ltall")
            head_cols = int(plan["wsum"][worder[0]]) // 16
            if 0 < head_cols < total_cols:
                nc.sync.dma_start(
                    out=ltall[:, :head_cols], in_=loc_ap[:, :head_cols]
                )
                nc.sync.dma_start(
                    out=ltall[:, head_cols:], in_=loc_ap[:, head_cols:]
                )
            else:
                nc.sync.dma_start(out=ltall[:], in_=loc_ap[:])

            psums = [
                pp.tile([128, D], f32, name=f"ps{g}", tag=f"ps{g}")
                for g in range(NPASS)
            ]

            for i, w in enumerate(worder):
                wsum_w = int(plan["wsum"][w])
                chk_w = int(plan["chk"][w])
                col0 = int(plan["woff"][w]) // 16

                g_t = gp.tile([128, chkmax, D], f16, name="g", tag="g")
                nc.gpsimd.dma_gather(
                    g_t[:, :chk_w, :],
                    tb_ap[w * W : (w + 1) * W, :],
                    ltall[:, col0 : col0 + wsum_w // 16],
                    wsum_w,
                    wsum_w,
                    D,
                    single_packet=False,
                    queue_num=i % NQ,
                )

                ccol0 = int(plan["coff"][w])
                for g in range(NPASS):
                    c0 = int(plan["cg0"][w, g])
                    c1 = int(plan["cg1"][w, g])
                    ncg = c1 - c0
                    A = ap_.tile([128, ncgmax, 128], f16, name="A", tag="A")
                    iota_bc = iotas[g][:].unsqueeze(1).broadcast_to(
                        [128, ncg, 128]
                    )
                    st_bc = stall[:, ccol0 + c0 : ccol0 + c1].unsqueeze(
                        2
                    ).broadcast_to([128, ncg, 128])
                    nc.vector.tensor_tensor(
                        A[:, :ncg, :], iota_bc, st_bc, Alu.is_equal
                    )
                    for c in range(ncg):
                        nc.tensor.matmul(
                            psums[g][:],
                            A[:, c, :],
                            g_t[:, c0 + c, :],
                            start=(i == 0 and c == 0),
                            stop=(i == NW - 1 and c == ncg - 1),
                        )

            for g in range(NPASS):
                outs = op.tile([128, D], f32, name="outs", tag="outs")
                nc.vector.tensor_copy(outs[:], psums[g][:])
                nc.sync.dma_start(
                    out=out_ap[g * 128 : (g + 1) * 128, :], in_=outs[:]
                )

    nc.compile()
    return nc


def _run(x, table, trace=False):
    from concourse.bass_utils import run_bass_kernel_spmd

    x_np = np.asarray(x)
    per_core = [
        _hash_buckets(x_np[c * BPC : (c + 1) * BPC]) for c in range(NCORES)
    ]
    cmax = np.max(
        [np.bincount(s, minlength=NSEG) for s, _, _ in per_core], axis=0
    )
    caps = (((np.maximum(cmax, 1) + 15) // 16) * 16).astype(np.int64)
    plan = _plan(caps)

    if "nc" not in _cache:
        _cache["nc"] = _build(caps, plan)
    nc = _cache["nc"]

    # fp16 table padded to NW*W rows so every gather window is full
    tb = np.zeros((NW * W, D), dtype=np.float16)
    tb[:V] = np.asarray(table).astype(np.float16)
    in_maps = []
    for c in range(NCORES):
        loc16, slotf = _layout(*per_core[c], plan)
        in_maps.append({"table": tb, "loc16": loc16, "slotf": slotf})
    res = run_bass_kernel_spmd(nc, in_maps, list(range(NCORES)), trace=trace)
    out = np.concatenate(
        [res.results[c]["out"] for c in range(NCORES)], axis=0
    ).astype(np.float32)
    return out, res


def kernel(x, table):
    out, _ = _run(x, table, trace=False)
    return out
